# revision 12
# baseline (speedup 1.0000x reference)
"""Wilson-Dirac operator on Trainium2, 8 NeuronCores, T-axis domain decomposition.

Lattice 24x24x24x48, complex64 field [X,Y,Z,T,3,4], gauge [4,X,Y,Z,T,3,3].
Sharding: T split into 8 slabs of 6, 1-site halos built host-side (periodic).
Host pre-scales the gauge field by -0.5 (the hopping-term prefactor), so the
device computes out = (m+4)*psi + sum_d U'_d(..) @ (proj_d psi)(x+delta_d)
with U' = -0.5*U.

Device layout per core (fp32, re/im interleaved innermost):
  fh   [X*Y, Z, TS+2, 3*4*2]    field slab with t-halo (t'=0 and TS+1)
  gh   [4, X*Y, Z, TS+1, 3*3*2] gauge slab (*-0.5), g index 0..TS <-> t0-1+g
  outp [X*Y, Z, TS, 3*4*2]

Compute: partition dim = (x,y) rows; half-spinor projection (h), per-site
color products on VectorE via 4-free-dim APs with broadcast (stride-0) dims,
b-sum in-place on the product tile, constant spin expansion via AP patterns.
Stencil shifts: x/y via shifted DMA loads, z via in-row AP offsets (wrap
split), t via inline halo. All ops keep <=4 free dims ((z,t) merged) --
the TRN2 walrus rejects more; it also allows only ONE sync-wait per
instruction, so excess waits on Tile's tail drain are spilled onto
appended NOPs (_fix_drain_waits).
"""

import numpy as np

# ---------------------------------------------------------------- constants
X = Y = Z = 24
T = 48
NCORES = 8
TS = T // NCORES          # slab interior
MASSP4 = 4.5              # mass + 4

# Direction spec. h_j = psi[j] + c_j * psi[B_j]  (j = 0,1)
# expansion: out[2] += d0 * m[e0], out[3] += d1 * m[e1]; out[0]+=m[0], out[1]+=m[1]
# backward direction: c -> -c, d -> -d.
DIRSPEC = {
    0: dict(B=(3, 2), c=(-1j, -1j), e=(1, 0), d=(+1j, +1j)),
    1: dict(B=(3, 2), c=(-1, +1),   e=(1, 0), d=(+1, -1)),
    2: dict(B=(2, 3), c=(-1j, +1j), e=(0, 1), d=(+1j, -1j)),
    3: dict(B=(2, 3), c=(+1, +1),   e=(0, 1), d=(+1, +1)),
}

_CACHE = {}


def _z_splits(z0, z1, dz, Zn):
    """out z-range [z0,z1) reading input at z+dz (periodic). -> [(oz, n, iz)]"""
    if dz == 0:
        return [(z0, z1 - z0, z0)]
    if dz == -1:
        if z0 == 0:
            out = [(0, 1, Zn - 1)]
            if z1 > 1:
                out.append((1, z1 - 1, 0))
            return out
        return [(z0, z1 - z0, z0 - 1)]
    if dz == +1:
        if z1 == Zn:
            out = []
            if Zn - 1 > z0:
                out.append((z0, Zn - 1 - z0, z0 + 1))
            out.append((Zn - 1, 1, 0))
            return out
        return [(z0, z1 - z0, z0 + 1)]
    raise ValueError(dz)


def _split_waits_json(raw: bytes) -> bytes:
    """This walrus build allows only ONE sync-wait per instruction (any
    class). Tile attaches several (multi-queue DMA deps, tail drain). Split:
    keep the last wait on the instruction, hoist the rest onto NoOps
    inserted immediately before it (same engine => same position in that
    engine's program order; semaphores are monotonic, so this is exact)."""
    import json
    bj = json.loads(raw)
    nid = [0]
    for fn in bj.get("functions", []):
        for bb in fn.get("blocks", []):
            insts = bb.get("instructions", [])
            out = []
            changed = False
            for inst in insts:
                si = inst.get("sync_info")
                ow = (si or {}).get("on_wait") or []
                if len(ow) > 1:
                    changed = True
                    for w in ow[:-1]:
                        nid[0] += 1
                        out.append({
                            "engine": inst["engine"],
                            "ins": [], "outs": [],
                            "name": f"WSPL-{nid[0]}",
                            "opcode": "NoOp",
                            "sync_info": {"on_update": [], "on_wait": [w]},
                        })
                    si["on_wait"] = [ow[-1]]
                out.append(inst)
            if changed:
                bb["instructions"] = out
    return json.dumps(bj).encode()


def _install_json_wait_fix():
    import concourse.bass as bass
    if getattr(bass.Bass, "_wd_wait_fix", False):
        return
    orig = bass.Bass.to_json_bytes

    def patched(self, *a, **k):
        return _split_waits_json(orig(self, *a, **k))

    bass.Bass.to_json_bytes = patched
    bass.Bass._wd_wait_fix = True


def build_module(Xl, Yl, Zl, TSl, n_zsplit=2, nxc_override=None):
    import concourse.bass as bass
    import concourse.mybir as mybir
    from concourse.ap import AP
    from concourse.mybir import AluOpType
    from concourse.tile import TileContext

    _install_json_wait_fix()

    F32 = mybir.dt.float32
    TH = TSl + 2
    TG = TSl + 1
    XY = Xl * Yl
    NSP = 24                    # field reals/site
    NSU = 18                    # gauge reals/site/mu

    nc = bass.Bass()
    fh = nc.declare_dram_parameter("fh", [XY, Zl, TH, NSP], F32, isOutput=False)
    gh = nc.declare_dram_parameter("gh", [4, XY, Zl, TG, NSU], F32, isOutput=False)
    outp = nc.declare_dram_parameter("outp", [XY, Zl, TSl, NSP], F32, isOutput=True)

    NXC = nxc_override or max(1, 128 // Yl)     # x-groups per chunk
    if n_zsplit > 1 and Zl % n_zsplit == 0:
        zh = Zl // n_zsplit
        zparts = [(i * zh, (i + 1) * zh) for i in range(n_zsplit)]
    else:
        zparts = [(0, Zl)]

    def sap(t, off, dims):
        return AP(t.tensor, t.offset + off, [list(t.ap[0])] + [list(d) for d in dims])

    with TileContext(nc) as tc:
        ctx_pool = tc.tile_pool(name="work", bufs=1)
        pool = ctx_pool.__enter__()
        V = nc.vector
        for x0 in range(0, Xl, NXC):
            nx = min(NXC, Xl - x0)
            R = nx * Yl
            r0 = x0 * Yl

            psi_al = pool.tile([R, Zl * TH * NSP], F32, tag="psi_al", bufs=2)
            out_t = pool.tile([R, Zl * TSl * NSP], F32, tag="out_t", bufs=2)
            # strides
            SA = dict(z=TH * NSP, t=NSP, c=8, s=2, ri=1)          # psi_al
            SS = dict(z=TSl * NSP, t=NSP, c=8, s=2, ri=1)         # psi shifted
            SO = dict(z=TSl * NSP, t=NSP, c=8, s=2, ri=1)         # out
            SH = dict(z=TSl * 12, t=12, j=6, b=2, ri=1)           # h
            SU = dict(z=TSl * NSU, t=NSU, a=6, b=2, ri=1)         # gauge tiles
            SP_ = dict(z=TSl * 72, t=72, j=36, a=12, b=4, th=2, tu=1)
            SM = dict(z=TSl * 12, t=12, j=6, a=2, ri=1)

            nc.sync.dma_start(out=psi_al[:], in_=fh[r0:r0 + R])

            def load_x(tag, src_tensor, mu, drow, tsl0, tsl1, nreals):
                tl = pool.tile([R, Zl * (tsl1 - tsl0) * nreals], F32, tag=tag, bufs=2)
                rs = (r0 + drow) % XY
                if src_tensor is fh:
                    src = lambda a, b: fh[a:b, :, tsl0:tsl1]
                else:
                    src = lambda a, b: gh[mu, a:b, :, tsl0:tsl1]
                if rs + R <= XY:
                    nc.sync.dma_start(out=tl[:], in_=src(rs, rs + R))
                else:
                    n1 = XY - rs
                    nc.sync.dma_start(out=tl[0:n1], in_=src(rs, XY))
                    nc.sync.dma_start(out=tl[n1:R], in_=src(0, R - n1))
                return tl

            def load_y(tag, src_tensor, mu, dy, tsl0, tsl1, nreals):
                tl = pool.tile([R, Zl * (tsl1 - tsl0) * nreals], F32, tag=tag, bufs=2)
                if src_tensor is fh:
                    src = lambda a, b: fh[a:b, :, tsl0:tsl1]
                else:
                    src = lambda a, b: gh[mu, a:b, :, tsl0:tsl1]
                for g in range(nx):
                    xa = x0 + g
                    if dy == +1:
                        nc.sync.dma_start(out=tl[g * Yl:g * Yl + Yl - 1],
                                          in_=src(xa * Yl + 1, xa * Yl + Yl))
                        nc.sync.dma_start(out=tl[g * Yl + Yl - 1:g * Yl + Yl],
                                          in_=src(xa * Yl, xa * Yl + 1))
                    else:
                        nc.sync.dma_start(out=tl[g * Yl + 1:g * Yl + Yl],
                                          in_=src(xa * Yl, xa * Yl + Yl - 1))
                        nc.sync.dma_start(out=tl[g * Yl:g * Yl + 1],
                                          in_=src(xa * Yl + Yl - 1, xa * Yl + Yl))
                return tl

            def load_g_al(mu, tsl0, tsl1):
                tl = pool.tile([R, Zl * TSl * NSU], F32, tag="g_al", bufs=6)
                nc.sync.dma_start(out=tl[:], in_=gh[mu, r0:r0 + R, :, tsl0:tsl1])
                return tl

            # mass term
            nc.scalar.mul(
                sap(out_t, 0, [[SO["z"], Zl], [NSP, TSl], [1, NSP]]),
                sap(psi_al, NSP, [[SA["z"], Zl], [NSP, TSl], [1, NSP]]),
                MASSP4)

            # ---------------- directions ----------------
            for mu in range(4):
                spec = DIRSPEC[mu]
                if mu < 3:
                    g_tiles = {False: load_g_al(mu, 1, TSl + 1)}
                    g_tiles[True] = g_tiles[False]
                else:
                    g_tiles = {True: load_g_al(3, 0, TSl),
                               False: load_g_al(3, 1, TSl + 1)}
                if mu == 0:
                    psi_f = load_x("psi_sh", fh, None, -Yl, 1, TSl + 1, NSP)
                    psi_b = load_x("psi_sh", fh, None, +Yl, 1, TSl + 1, NSP)
                    g_f = load_x("g_sh", gh, 0, -Yl, 1, TSl + 1, NSU)
                elif mu == 1:
                    psi_f = load_y("psi_sh", fh, None, -1, 1, TSl + 1, NSP)
                    psi_b = load_y("psi_sh", fh, None, +1, 1, TSl + 1, NSP)
                    g_f = load_y("g_sh", gh, 1, -1, 1, TSl + 1, NSU)

                for sgn in (+1, -1):   # +1 = forward (U^dag(x-mu)), -1 = backward
                    fwd = sgn == +1
                    cj = spec["c"] if fwd else tuple(-v for v in spec["c"])
                    dj = spec["d"] if fwd else tuple(-v for v in spec["d"])

                    if mu <= 1:
                        psit, dzp, toffp, SPS = (psi_f if fwd else psi_b), 0, 0, SS
                    elif mu == 2:
                        psit, dzp, toffp, SPS = psi_al, (-1 if fwd else +1), NSP, SA
                    else:
                        psit, dzp, toffp, SPS = psi_al, 0, (0 if fwd else 2 * NSP), SA

                    # --- projection: h tile (full z); ops per (j, ri): 3 free dims
                    ht = pool.tile([R, Zl * TSl * 12], F32, tag="h", bufs=1)
                    for j in (0, 1):
                        A, B, c = j, spec["B"][j], cj[j]
                        for (oz, nz, iz) in _z_splits(0, Zl, dzp, Zl):
                            hbase = oz * SH["z"] + j * SH["j"]
                            pb = iz * SPS["z"] + toffp
                            # merged (z,t) only valid for SS tiles; keep (z,t)
                            zt = [[SPS["z"], nz], [SPS["t"], TSl]]
                            hzt = [[SH["z"], nz], [SH["t"], TSl]]
                            if c.imag == 0.0:
                                op = AluOpType.add if c.real > 0 else AluOpType.subtract
                                for ri in (0, 1):
                                    V.tensor_tensor(
                                        sap(ht, hbase + ri, hzt + [[SH["b"], 3]]),
                                        sap(psit, pb + A * 2 + ri, zt + [[SPS["c"], 3]]),
                                        sap(psit, pb + B * 2 + ri, zt + [[SPS["c"], 3]]),
                                        op)
                            else:
                                sg = 1.0 if c.imag > 0 else -1.0
                                V.tensor_tensor(
                                    sap(ht, hbase, hzt + [[SH["b"], 3]]),
                                    sap(psit, pb + A * 2, zt + [[SPS["c"], 3]]),
                                    sap(psit, pb + B * 2 + 1, zt + [[SPS["c"], 3]]),
                                    AluOpType.subtract if sg > 0 else AluOpType.add)
                                V.tensor_tensor(
                                    sap(ht, hbase + 1, hzt + [[SH["b"], 3]]),
                                    sap(psit, pb + A * 2 + 1, zt + [[SPS["c"], 3]]),
                                    sap(psit, pb + B * 2, zt + [[SPS["c"], 3]]),
                                    AluOpType.add if sg > 0 else AluOpType.subtract)

                    # --- gauge source (t-shift folded into tile choice for mu3)
                    if fwd and mu == 0:
                        gt, dzu = g_f, 0
                    elif fwd and mu == 1:
                        gt, dzu = g_f, 0
                    elif fwd and mu == 2:
                        gt, dzu = g_tiles[True], -1
                    else:
                        gt, dzu = g_tiles[fwd], 0
                    ast, bst = (SU["b"], SU["a"]) if fwd else (SU["a"], SU["b"])

                    for (zl0, zl1) in zparts:
                        hz = zl1 - zl0
                        pt = pool.tile([R, hz * TSl * 72], F32, tag="P", bufs=1)
                        mt = pool.tile([R, hz * TSl * 12], F32, tag="m", bufs=1)

                        # --- products: per (j,th,tu): out (zt,a,b), 3 free dims
                        for j in (0, 1):
                            for th in (0, 1):
                                for tu in (0, 1):
                                    for (oz, nz, iz) in _z_splits(zl0, zl1, dzu, Zl):
                                        po = (oz - zl0) * SP_["z"] + j * SP_["j"] + th * SP_["th"] + tu
                                        V.tensor_tensor(
                                            sap(pt, po, [[SP_["t"], nz * TSl], [SP_["a"], 3], [SP_["b"], 3]]),
                                            sap(gt, iz * SU["z"] + tu, [[SU["t"], nz * TSl], [ast, 3], [bst, 3]]),
                                            sap(ht, oz * SH["z"] + j * SH["j"] + th, [[SH["t"], nz * TSl], [0, 3], [SH["b"], 3]]),
                                            AluOpType.mult)

                        # --- b-sum in place on P: P[b0] += P[b1]; P[b0] += P[b2]
                        bdims = [[SP_["t"], hz * TSl], [SP_["a"], 6], [1, 4]]
                        V.tensor_tensor(sap(pt, 0, bdims), sap(pt, 0, bdims),
                                        sap(pt, SP_["b"], bdims), AluOpType.add)
                        V.tensor_tensor(sap(pt, 0, bdims), sap(pt, 0, bdims),
                                        sap(pt, 2 * SP_["b"], bdims), AluOpType.add)
                        # --- combine: m_re = P_rr +- P_ii ; m_im = P_ir -+ P_ri
                        cdims = [[SP_["t"], hz * TSl], [SP_["a"], 6]]
                        mdims = [[SM["t"], hz * TSl], [SM["a"], 6]]
                        V.tensor_tensor(sap(mt, 0, mdims), sap(pt, 0, cdims), sap(pt, 3, cdims),
                                        AluOpType.add if fwd else AluOpType.subtract)
                        V.tensor_tensor(sap(mt, 1, mdims), sap(pt, 2, cdims), sap(pt, 1, cdims),
                                        AluOpType.subtract if fwd else AluOpType.add)

                        # --- expansion into out_t z-slice; per-s ops, 3 free dims
                        ob = zl0 * SO["z"]
                        ozt = [[NSP, hz * TSl]]
                        mzt = [[SM["t"], hz * TSl]]
                        # s=0,1: out[s] += m[j=s]  (coefficient +1)
                        for s in (0, 1):
                            os_ = sap(out_t, ob + s * SO["s"], ozt + [[SO["c"], 3], [1, 2]])
                            V.tensor_tensor(os_, os_,
                                            sap(mt, s * SM["j"], mzt + [[SM["a"], 3], [1, 2]]),
                                            AluOpType.add)
                        # s=2,3: out[s] += d * m[e]
                        for si_, (ei, dv) in enumerate(zip(spec["e"], dj)):
                            sb = ob + (2 + si_) * SO["s"]
                            if dv.imag == 0.0:
                                op = AluOpType.add if dv.real > 0 else AluOpType.subtract
                                os_ = sap(out_t, sb, ozt + [[SO["c"], 3], [1, 2]])
                                V.tensor_tensor(os_, os_,
                                                sap(mt, ei * SM["j"], mzt + [[SM["a"], 3], [1, 2]]), op)
                            else:
                                sg = 1.0 if dv.imag > 0 else -1.0
                                ore = sap(out_t, sb, ozt + [[SO["c"], 3]])
                                V.tensor_tensor(ore, ore,
                                                sap(mt, ei * SM["j"] + 1, mzt + [[SM["a"], 3]]),
                                                AluOpType.subtract if sg > 0 else AluOpType.add)
                                oim = sap(out_t, sb + 1, ozt + [[SO["c"], 3]])
                                V.tensor_tensor(oim, oim,
                                                sap(mt, ei * SM["j"], mzt + [[SM["a"], 3]]),
                                                AluOpType.add if sg > 0 else AluOpType.subtract)

            nc.sync.dma_start(out=outp[r0:r0 + R], in_=out_t[:])
        ctx_pool.__exit__(None, None, None)
    return nc


# ---------------------------------------------------------------- host side
def _prep_core_inputs(fv, gv, t0, Xl, Yl, Zl, Tl, TSl):
    idx = [(t0 - 1) % Tl] + [(t0 + i) % Tl for i in range(TSl)] + [(t0 + TSl) % Tl]
    fhn = np.ascontiguousarray(fv[:, :, :, idx].reshape(Xl * Yl, Zl, TSl + 2, 24))
    idg = [(t0 - 1 + i) % Tl for i in range(TSl + 1)]
    ghn = np.ascontiguousarray(gv[:, :, :, :, idg].reshape(4, Xl * Yl, Zl, TSl + 1, 18))
    ghn *= -0.5
    return fhn, ghn


def kernel(field, gauge_field):
    from concourse.bass_utils import run_bass_kernel_spmd

    key = "full"
    if key not in _CACHE:
        _CACHE[key] = build_module(X, Y, Z, TS)
    nc = _CACHE[key]

    fv = np.ascontiguousarray(field).view(np.float32).reshape(X, Y, Z, T, 3, 4, 2)
    gv = np.ascontiguousarray(gauge_field).view(np.float32).reshape(4, X, Y, Z, T, 3, 3, 2)

    in_maps = []
    for k in range(NCORES):
        fhn, ghn = _prep_core_inputs(fv, gv, k * TS, X, Y, Z, T, TS)
        in_maps.append({"fh": fhn, "gh": ghn})

    res = run_bass_kernel_spmd(nc, in_maps, list(range(NCORES))).results

    out = np.empty((X, Y, Z, T, 3, 4), np.complex64)
    for k in range(NCORES):
        o = res[k]["outp"].reshape(X, Y, Z, TS, 3, 4, 2)
        out[:, :, :, k * TS:(k + 1) * TS] = o[..., 0] + 1j * o[..., 1]
    return out


# revision 14
# speedup vs baseline: 1.1497x; 1.1497x over previous
"""Wilson-Dirac operator on Trainium2, 8 NeuronCores, T-axis domain decomposition.

Lattice 24x24x24x48, complex64 field [X,Y,Z,T,3,4], gauge [4,X,Y,Z,T,3,3].
Sharding: T split into 8 slabs of 6, 1-site halos built host-side (periodic).
Host pre-scales the gauge field by -0.5 (the hopping prefactor) and ships it
twice in direction-specific layouts so every VectorE operand streams with
innermost stride <= 2 elements (strides >= 12B measured 1.25-1.6x slower):

  fh    [X*Y, Z, TS+2, s4, c3, ri2]  field slab, t-halo inline
  ghb   [4, X*Y, Z, TS+1, ri2, b3, a3]  -0.5*U[a,b] at [ri][b][a]  (backward)
  ghf   [4, X*Y, Z, TS+1, ri2, a3, b3]  -0.5*U[a,b] at [ri][a][b]  (forward,
        read transposed as U[b_out,a_out] with steps (3,1))
  outp  [X*Y, Z, TS, s4, c3, ri2]

Compute: partition = (x,y) rows. Half-spinor projection h (j,b,ri), per-site
color products into P (j,th,tu,b,a), in-place b-sum, Re/Im combine into
m (j,a,ri), spin expansion into out accumulator. Shifts: x/y via shifted DMA
row loads, z via in-row AP offsets (periodic wrap split), t via inline halo.
All engine-op APs keep <= 3 free dims (walrus TENSOR3D limit) and the
one-sync-wait-per-instruction walrus limit is handled by splitting waits
onto NoOps at BIR-json level (_split_waits_json)."""

import numpy as np

# ---------------------------------------------------------------- constants
X = Y = Z = 24
T = 48
NCORES = 8
TS = T // NCORES
MASSP4 = 4.5

# h_j = psi[j] + c_j * psi[B_j]; expansion: out[0]+=m[0], out[1]+=m[1],
# out[2] += d0*m[e0], out[3] += d1*m[e1].  Backward: c -> -c, d -> -d.
DIRSPEC = {
    0: dict(B=(3, 2), c=(-1j, -1j), e=(1, 0), d=(+1j, +1j)),
    1: dict(B=(3, 2), c=(-1, +1),   e=(1, 0), d=(+1, -1)),
    2: dict(B=(2, 3), c=(-1j, +1j), e=(0, 1), d=(+1j, -1j)),
    3: dict(B=(2, 3), c=(+1, +1),   e=(0, 1), d=(+1, +1)),
}

_CACHE = {}


def _z_splits(z0, z1, dz, Zn):
    """out z-range [z0,z1) reading input at z+dz (periodic). -> [(oz, n, iz)]"""
    if dz == 0:
        return [(z0, z1 - z0, z0)]
    if dz == -1:
        if z0 == 0:
            out = [(0, 1, Zn - 1)]
            if z1 > 1:
                out.append((1, z1 - 1, 0))
            return out
        return [(z0, z1 - z0, z0 - 1)]
    if dz == +1:
        if z1 == Zn:
            out = []
            if Zn - 1 > z0:
                out.append((z0, Zn - 1 - z0, z0 + 1))
            out.append((Zn - 1, 1, 0))
            return out
        return [(z0, z1 - z0, z0 + 1)]
    raise ValueError(dz)


def _split_waits_json(raw: bytes) -> bytes:
    """Walrus here allows only ONE sync-wait per instruction. Keep the last
    wait on the instruction, hoist the rest onto NoOps inserted immediately
    before it (same engine, semaphores monotonic => exact)."""
    import json
    bj = json.loads(raw)
    nid = 0
    for fn in bj.get("functions", []):
        for bb in fn.get("blocks", []):
            out = []
            changed = False
            for inst in bb.get("instructions", []):
                si = inst.get("sync_info")
                ow = (si or {}).get("on_wait") or []
                if len(ow) > 1:
                    changed = True
                    for w in ow[:-1]:
                        nid += 1
                        out.append({
                            "engine": inst["engine"], "ins": [], "outs": [],
                            "name": f"WSPL-{nid}", "opcode": "NoOp",
                            "sync_info": {"on_update": [], "on_wait": [w]},
                        })
                    si["on_wait"] = [ow[-1]]
                out.append(inst)
            if changed:
                bb["instructions"] = out
    return json.dumps(bj).encode()


def _install_json_wait_fix():
    import concourse.bass as bass
    if getattr(bass.Bass, "_wd_wait_fix", False):
        return
    orig = bass.Bass.to_json_bytes

    def patched(self, *a, **k):
        return _split_waits_json(orig(self, *a, **k))

    bass.Bass.to_json_bytes = patched
    bass.Bass._wd_wait_fix = True


def build_module(Xl, Yl, Zl, TSl, n_zsplit=2, nxc_override=None):
    import concourse.bass as bass
    import concourse.mybir as mybir
    from concourse.ap import AP
    from concourse.mybir import AluOpType
    from concourse.tile import TileContext

    _install_json_wait_fix()

    F32 = mybir.dt.float32
    TH = TSl + 2
    TG = TSl + 1
    XY = Xl * Yl
    NSP = 24
    NSU = 18

    nc = bass.Bass()
    fh = nc.declare_dram_parameter("fh", [XY, Zl, TH, NSP], F32, isOutput=False)
    ghb = nc.declare_dram_parameter("ghb", [4, XY, Zl, TG, NSU], F32, isOutput=False)
    ghf = nc.declare_dram_parameter("ghf", [4, XY, Zl, TG, NSU], F32, isOutput=False)
    outp = nc.declare_dram_parameter("outp", [XY, Zl, TSl, NSP], F32, isOutput=True)

    NXC = nxc_override or max(1, 128 // Yl)
    if n_zsplit > 1 and Zl % n_zsplit == 0:
        zh = Zl // n_zsplit
        zparts = [(i * zh, (i + 1) * zh) for i in range(n_zsplit)]
    else:
        zparts = [(0, Zl)]

    def sap(t, off, dims):
        return AP(t.tensor, t.offset + off, [list(t.ap[0])] + [list(d) for d in dims])

    with TileContext(nc) as tc:
        ctx_pool = tc.tile_pool(name="work", bufs=1)
        pool = ctx_pool.__enter__()
        V = nc.vector
        for x0 in range(0, Xl, NXC):
            nx = min(NXC, Xl - x0)
            R = nx * Yl
            r0 = x0 * Yl

            psi_al = pool.tile([R, Zl * TH * NSP], F32, tag="psi_al", bufs=2)
            out_t = pool.tile([R, Zl * TSl * NSP], F32, tag="out_t", bufs=2)
            # strides (field site-block = (s4, c3, ri2))
            SA = dict(z=TH * NSP, t=NSP, s=6, c=2, ri=1)   # psi_al
            SS = dict(z=TSl * NSP, t=NSP, s=6, c=2, ri=1)  # psi shifted
            SO = dict(z=TSl * NSP, t=NSP, s=6, c=2, ri=1)  # out
            SH = dict(z=TSl * 12, t=12, j=6, b=2, ri=1)    # h
            SU = dict(z=TSl * NSU, t=NSU, ri=9, r3=3, c1=1)  # gauge tiles
            SP_ = dict(z=TSl * 72, t=72, j=36, th=18, tu=9, b=3, a=1)
            SM = dict(z=TSl * 12, t=12, j=6, a=2, ri=1)    # m

            nc.sync.dma_start(out=psi_al[:], in_=fh[r0:r0 + R])

            def load_x(tag, src_tensor, mu, drow, tsl0, tsl1, nreals):
                tl = pool.tile([R, Zl * (tsl1 - tsl0) * nreals], F32, tag=tag,
                               bufs=(8 if tag == "g_al" else 2))
                rs = (r0 + drow) % XY
                if src_tensor is None:
                    src = lambda a, b: fh[a:b, :, tsl0:tsl1]
                else:
                    src = lambda a, b: src_tensor[mu, a:b, :, tsl0:tsl1]
                if rs + R <= XY:
                    nc.sync.dma_start(out=tl[:], in_=src(rs, rs + R))
                else:
                    n1 = XY - rs
                    nc.sync.dma_start(out=tl[0:n1], in_=src(rs, XY))
                    nc.sync.dma_start(out=tl[n1:R], in_=src(0, R - n1))
                return tl

            def load_y(tag, src_tensor, mu, dy, tsl0, tsl1, nreals):
                tl = pool.tile([R, Zl * (tsl1 - tsl0) * nreals], F32, tag=tag,
                               bufs=(8 if tag == "g_al" else 2))
                if src_tensor is None:
                    src = lambda a, b: fh[a:b, :, tsl0:tsl1]
                else:
                    src = lambda a, b: src_tensor[mu, a:b, :, tsl0:tsl1]
                for g in range(nx):
                    xa = x0 + g
                    if dy == +1:
                        nc.sync.dma_start(out=tl[g * Yl:g * Yl + Yl - 1],
                                          in_=src(xa * Yl + 1, xa * Yl + Yl))
                        nc.sync.dma_start(out=tl[g * Yl + Yl - 1:g * Yl + Yl],
                                          in_=src(xa * Yl, xa * Yl + 1))
                    else:
                        nc.sync.dma_start(out=tl[g * Yl + 1:g * Yl + Yl],
                                          in_=src(xa * Yl, xa * Yl + Yl - 1))
                        nc.sync.dma_start(out=tl[g * Yl:g * Yl + 1],
                                          in_=src(xa * Yl + Yl - 1, xa * Yl + Yl))
                return tl

            def load_g(src_tensor, mu, tsl0, tsl1):
                tl = pool.tile([R, Zl * TSl * NSU], F32, tag="g_al", bufs=8)
                nc.sync.dma_start(out=tl[:], in_=src_tensor[mu, r0:r0 + R, :, tsl0:tsl1])
                return tl

            # mass term (ACT)
            nc.scalar.mul(
                sap(out_t, 0, [[SO["z"], Zl], [NSP, TSl], [1, NSP]]),
                sap(psi_al, NSP, [[SA["z"], Zl], [NSP, TSl], [1, NSP]]),
                MASSP4)

            for mu in range(4):
                spec = DIRSPEC[mu]
                # gauge tiles: fwd from ghf (transposed-read layout), bwd from ghb
                if mu == 0:
                    g_fwd = load_x("g_al", ghf, 0, -Yl, 1, TSl + 1, NSU)
                    g_bwd = load_g(ghb, 0, 1, TSl + 1)
                    psi_f = load_x("psi_sh", None, None, -Yl, 1, TSl + 1, NSP)
                    psi_b = load_x("psi_sh", None, None, +Yl, 1, TSl + 1, NSP)
                elif mu == 1:
                    g_fwd = load_y("g_al", ghf, 1, -1, 1, TSl + 1, NSU)
                    g_bwd = load_g(ghb, 1, 1, TSl + 1)
                    psi_f = load_y("psi_sh", None, None, -1, 1, TSl + 1, NSP)
                    psi_b = load_y("psi_sh", None, None, +1, 1, TSl + 1, NSP)
                elif mu == 2:
                    g_fwd = load_g(ghf, 2, 1, TSl + 1)
                    g_bwd = load_g(ghb, 2, 1, TSl + 1)
                else:
                    g_fwd = load_g(ghf, 3, 0, TSl)
                    g_bwd = load_g(ghb, 3, 1, TSl + 1)

                for sgn in (+1, -1):
                    fwd = sgn == +1
                    cj = spec["c"] if fwd else tuple(-v for v in spec["c"])
                    dj = spec["d"] if fwd else tuple(-v for v in spec["d"])

                    if mu <= 1:
                        psit, dzp, toffp, SPS = (psi_f if fwd else psi_b), 0, 0, SS
                    elif mu == 2:
                        psit, dzp, toffp, SPS = psi_al, (-1 if fwd else +1), NSP, SA
                    else:
                        psit, dzp, toffp, SPS = psi_al, 0, (0 if fwd else 2 * NSP), SA

                    # --- projection into h (j, b, ri); psi innermost (c,ri)
                    ht = pool.tile([R, Zl * TSl * 12], F32, tag="h", bufs=1)
                    for j in (0, 1):
                        A, B, c = j, spec["B"][j], cj[j]
                        for (oz, nz, iz) in _z_splits(0, Zl, dzp, Zl):
                            hbase = oz * SH["z"] + j * SH["j"]
                            pb = iz * SPS["z"] + toffp
                            zt = [[SPS["z"], nz], [SPS["t"], TSl]]
                            hzt = [[SH["z"], nz], [SH["t"], TSl]]
                            if c.imag == 0.0:
                                op = AluOpType.add if c.real > 0 else AluOpType.subtract
                                V.tensor_tensor(
                                    sap(ht, hbase, hzt + [[1, 6]]),
                                    sap(psit, pb + A * 6, zt + [[1, 6]]),
                                    sap(psit, pb + B * 6, zt + [[1, 6]]),
                                    op)
                            else:
                                sg = 1.0 if c.imag > 0 else -1.0
                                # h_re = psiA_re - sg*psiB_im ; h_im = psiA_im + sg*psiB_re
                                V.tensor_tensor(
                                    sap(ht, hbase, hzt + [[SH["b"], 3]]),
                                    sap(psit, pb + A * 6, zt + [[SPS["c"], 3]]),
                                    sap(psit, pb + B * 6 + 1, zt + [[SPS["c"], 3]]),
                                    AluOpType.subtract if sg > 0 else AluOpType.add)
                                V.tensor_tensor(
                                    sap(ht, hbase + 1, hzt + [[SH["b"], 3]]),
                                    sap(psit, pb + A * 6 + 1, zt + [[SPS["c"], 3]]),
                                    sap(psit, pb + B * 6, zt + [[SPS["c"], 3]]),
                                    AluOpType.add if sg > 0 else AluOpType.subtract)

                    gt = g_fwd if fwd else g_bwd
                    dzu = -1 if (fwd and mu == 2) else 0

                    for (zl0, zl1) in zparts:
                        hz = zl1 - zl0
                        pt = pool.tile([R, hz * TSl * 72], F32, tag="P", bufs=1)
                        mt = pool.tile([R, hz * TSl * 12], F32, tag="m", bufs=1)

                        # --- products: per (j,th,tu): P[zt,(b,a)] = U' * h
                        # out/in0 innermost stride 1, in1 broadcast over a
                        for j in (0, 1):
                            for th in (0, 1):
                                for tu in (0, 1):
                                    for (oz, nz, iz) in _z_splits(zl0, zl1, dzu, Zl):
                                        po = (oz - zl0) * SP_["z"] + j * SP_["j"] + th * SP_["th"] + tu * SP_["tu"]
                                        V.tensor_tensor(
                                            sap(pt, po, [[SP_["t"], nz * TSl], [SP_["b"], 3], [1, 3]]),
                                            sap(gt, iz * SU["z"] + tu * SU["ri"], [[SU["t"], nz * TSl], [3, 3], [1, 3]]),
                                            sap(ht, oz * SH["z"] + j * SH["j"] + th, [[SH["t"], nz * TSl], [SH["b"], 3], [0, 3]]),
                                            AluOpType.mult)

                        # --- b-sum in place: P[b0] += P[b1]; P[b0] += P[b2]
                        bdims = [[SP_["t"], hz * TSl], [SP_["tu"], 8], [1, 3]]
                        V.tensor_tensor(sap(pt, 0, bdims), sap(pt, 0, bdims),
                                        sap(pt, SP_["b"], bdims), AluOpType.add)
                        V.tensor_tensor(sap(pt, 0, bdims), sap(pt, 0, bdims),
                                        sap(pt, 2 * SP_["b"], bdims), AluOpType.add)
                        # --- combine into m (j, a, ri):
                        # m_re = P[rr] +- P[ii]; m_im = P[ir] -+ P[ri]
                        cdims = [[SP_["t"], hz * TSl], [SP_["j"], 2], [1, 3]]
                        mdims = [[SM["t"], hz * TSl], [SM["j"], 2], [SM["a"], 3]]
                        RR, II = 0, SP_["th"] + SP_["tu"]
                        IR, RI = SP_["th"], SP_["tu"]
                        V.tensor_tensor(sap(mt, 0, mdims), sap(pt, RR, cdims), sap(pt, II, cdims),
                                        AluOpType.add if fwd else AluOpType.subtract)
                        V.tensor_tensor(sap(mt, 1, mdims), sap(pt, IR, cdims), sap(pt, RI, cdims),
                                        AluOpType.subtract if fwd else AluOpType.add)

                        # --- expansion into out_t (s,c,ri layout; (c,ri)=[1,6])
                        ob = zl0 * SO["z"]
                        ozt = [[NSP, hz * TSl]]
                        mzt = [[SM["t"], hz * TSl]]
                        for s in (0, 1):
                            os_ = sap(out_t, ob + s * SO["s"], ozt + [[1, 6]])
                            V.tensor_tensor(os_, os_, sap(mt, s * SM["j"], mzt + [[1, 6]]),
                                            AluOpType.add)
                        for si_, (ei, dv) in enumerate(zip(spec["e"], dj)):
                            sb = ob + (2 + si_) * SO["s"]
                            if dv.imag == 0.0:
                                op = AluOpType.add if dv.real > 0 else AluOpType.subtract
                                os_ = sap(out_t, sb, ozt + [[1, 6]])
                                V.tensor_tensor(os_, os_, sap(mt, ei * SM["j"], mzt + [[1, 6]]), op)
                            else:
                                sg = 1.0 if dv.imag > 0 else -1.0
                                # out_re += -sg*m_im ; out_im += sg*m_re
                                ore = sap(out_t, sb, ozt + [[SO["c"], 3]])
                                V.tensor_tensor(ore, ore,
                                                sap(mt, ei * SM["j"] + 1, mzt + [[SM["a"], 3]]),
                                                AluOpType.subtract if sg > 0 else AluOpType.add)
                                oim = sap(out_t, sb + 1, ozt + [[SO["c"], 3]])
                                V.tensor_tensor(oim, oim,
                                                sap(mt, ei * SM["j"], mzt + [[SM["a"], 3]]),
                                                AluOpType.add if sg > 0 else AluOpType.subtract)

            nc.sync.dma_start(out=outp[r0:r0 + R], in_=out_t[:])
        ctx_pool.__exit__(None, None, None)
    return nc


# ---------------------------------------------------------------- host side
def _prep_core_inputs(fv, gv, t0, Xl, Yl, Zl, Tl, TSl):
    """fv: [X,Y,Z,T,3,4,2] f32 view (c,s,ri). gv: [4,X,Y,Z,T,3,3,2] (a,b,ri).
    Returns fh [XY,Z,TH,(s,c,ri)], ghb [...,(ri,b,a)], ghf [...,(ri,a,b)],
    gauge pre-scaled by -0.5."""
    idx = [(t0 - 1) % Tl] + [(t0 + i) % Tl for i in range(TSl)] + [(t0 + TSl) % Tl]
    f = fv[:, :, :, idx]                       # [X,Y,Z,TH,c,s,ri]
    f = f.transpose(0, 1, 2, 3, 5, 4, 6)       # -> (s,c,ri)
    fhn = np.ascontiguousarray(f).reshape(Xl * Yl, Zl, TSl + 2, 24)
    idg = [(t0 - 1 + i) % Tl for i in range(TSl + 1)]
    g = gv[:, :, :, :, idg]                    # [4,X,Y,Z,TG,a,b,ri]
    ghfn = np.ascontiguousarray(g.transpose(0, 1, 2, 3, 4, 7, 5, 6))  # (ri,a,b)
    ghbn = np.ascontiguousarray(g.transpose(0, 1, 2, 3, 4, 7, 6, 5))  # (ri,b,a)
    ghfn *= -0.5
    ghbn *= -0.5
    return (fhn, ghfn.reshape(4, Xl * Yl, Zl, TSl + 1, 18),
            ghbn.reshape(4, Xl * Yl, Zl, TSl + 1, 18))


def _out_to_complex(o, Xl, Yl, Zl, TSl):
    o = o.reshape(Xl, Yl, Zl, TSl, 4, 3, 2)    # (s,c,ri)
    o = o.transpose(0, 1, 2, 3, 5, 4, 6)       # -> (c,s,ri)
    return o[..., 0] + 1j * o[..., 1]


def kernel(field, gauge_field):
    from concourse.bass_utils import run_bass_kernel_spmd

    key = "full"
    if key not in _CACHE:
        _CACHE[key] = build_module(X, Y, Z, TS)
    nc = _CACHE[key]

    fv = np.ascontiguousarray(field).view(np.float32).reshape(X, Y, Z, T, 3, 4, 2)
    gv = np.ascontiguousarray(gauge_field).view(np.float32).reshape(4, X, Y, Z, T, 3, 3, 2)

    in_maps = []
    for k in range(NCORES):
        fhn, ghfn, ghbn = _prep_core_inputs(fv, gv, k * TS, X, Y, Z, T, TS)
        in_maps.append({"fh": fhn, "ghf": ghfn, "ghb": ghbn})

    res = run_bass_kernel_spmd(nc, in_maps, list(range(NCORES))).results

    out = np.empty((X, Y, Z, T, 3, 4), np.complex64)
    for k in range(NCORES):
        out[:, :, :, k * TS:(k + 1) * TS] = _out_to_complex(
            res[k]["outp"], X, Y, Z, TS)
    return out
